# revision 1
# baseline (speedup 1.0000x reference)
"""Trainium2 Bass kernel for BCNet-style fused block.

Reference computation (per batch b):
    v_ = relu(v @ Wv.T + bv)            # [B, NO, H]
    q_ = relu(q @ Wq.T + bq)            # [B, Q,  H]
    qw = einsum("bqh,q->bh", q_, wh)    # [B, H]
    logits = v_ * qw[:, None, :] + bh   # [B, NO, H]
    out = logits @ W2.T + b2            # [B, NO, VD]

Strategy: pure data parallel over batch (16 per core x 8 cores), weights
replicated. All matmuls run in bf16 with fp32 PSUM accumulation; weights /
activations are pre-cast and pre-transposed on host so the device streams
them in matmul-native layouts with no on-chip transposes.

Per-core dataflow (H or VD on the partition dim throughout):
  A: q_T = relu(WqT.T @ qT + bq)    -> *wh -> segment-reduce over Q -> qw_T
  B: v_T = relu(WvT.T @ vT + bv)    -> logits_T = v_T * qw_T (broadcast)
  C: out_T = W2T.T @ logits_T + b2eff  (bh folded into b2eff on host)
Output is produced transposed [VD, rows]; host transposes back.

Scheduling notes (engines execute their streams in order; DMA transfers are
effectively serialized at ~350GB/s, dma_start dispatch ~0.6us per queue):
- Matmul loops run k-outer over blocks of concurrent PSUM groups so each
  arriving weight chunk unlocks work in every in-flight group.
- Weights load as a few large DMAs (one per column block, all k-tiles in
  one 3D access pattern), emitted in PE consumption order.
- The first B block (m 0-3, both n halves) is issued BEFORE phase A: it
  only needs vT + the first WvT column block, so ~15us of real PE work
  runs while the WqT stream is still on the bus. Its evictions are
  split: the ACT relus run immediately (freeing the PSUM banks so phase
  A can use 8-group halves); the qw multiplies are deferred to after A.
- Bus order is hand-paced to PE consumption: vT/WvT k-chunks first,
  consts and qT (needed only by evictions / phase A) after them.
- ~100 tiny warmup matmuls on a zeroed tile fill the initial DMA wait so
  the PE clock (HAM) is already un-throttled when the real stream starts.
"""

import os
import sys

import numpy as np

for _p in ("/opt/trn_rl_repo", "/root/.axon_site/_ro/trn_rl_repo"):
    if os.path.isdir(_p) and _p not in sys.path:
        sys.path.insert(0, _p)

import ml_dtypes

import concourse.bacc as bacc
import concourse.bass as bass
import concourse.mybir as mybir
import concourse.tile as tile
from concourse.bass_utils import run_bass_kernel_spmd

B, NO, Q = 128, 36, 14
VD, QD, H = 2048, 1024, 2048
NCORES = 8
BS = B // NCORES          # 16 batches per core
NROW = BS * NO            # 576 v-rows per core
QROW = BS * Q             # 224 q-rows per core
P = 128
NT = 288                  # n-tile for matmuls 1/3 (2 tiles of 8 batches * 36)
NN = NROW // NT           # 2
BPT = NT // NO            # 8 batches per n-tile
KV = VD // P              # 16 contraction tiles for matmul 1
KQ = QD // P              # 8  contraction tiles for matmul 2
MH = H // P               # 16 output h-tiles
KH = H // P               # 16 contraction tiles for matmul 3
MV = VD // P              # 16 output vd-tiles

F32 = mybir.dt.float32
BF16 = mybir.dt.bfloat16
BF16_NP = ml_dtypes.bfloat16


def _build_program(opts=None):
    o = dict(
        wq_split=2,   # column blocks for WqT (phase-A pacing granularity)
        wv_split=4,   # column blocks for WvT (must match phase-B m-blocks of 4)
        w2_split=4,   # column blocks for W2T (16KB/partition slot, matches wv)
        warmup=100,   # PE warmup matmuls before the first real matmul
        out_split=True,   # one output DMA per (m, n) instead of per m
        wq_eng="sync",    # queue for the WqT stream
        out_eng="sync",   # issuing engine for output DMAs
        wv0_chunks=8,     # k-chunks for the first WvT column block
        tail_split=True,  # half-width final output group (shorter tail)
        b0_first=True,  # issue B-block0 (m0-3, n0) before phase A
        psum_bufs=8,
    )
    if opts:
        o.update(opts)

    nc = bacc.Bacc("TRN2", target_bir_lowering=False, debug=False, num_devices=NCORES)

    vT = nc.dram_tensor("vT", [P, NN * KV * NT], BF16, kind="ExternalInput").ap()
    qT = nc.dram_tensor("qT", [P, KQ * QROW], BF16, kind="ExternalInput").ap()
    WvT = nc.dram_tensor("WvT", [VD, H], BF16, kind="ExternalInput").ap()
    WqT = nc.dram_tensor("WqT", [QD, H], BF16, kind="ExternalInput").ap()
    W2T = nc.dram_tensor("W2T", [H, VD], BF16, kind="ExternalInput").ap()
    constC = nc.dram_tensor("constC", [P, 3 * 16 + QROW], F32,
                            kind="ExternalInput").ap()
    outT = nc.dram_tensor("outT", [VD, NROW], F32, kind="ExternalOutput").ap()

    # DRAM views with k-tiles split out
    qT_r = qT.rearrange("p (k c) -> p k c", k=KQ)
    vT_r = vT.rearrange("p (n k c) -> p n k c", n=NN, k=KV)
    WqT_r = WqT.rearrange("(k p) c -> p k c", p=P)
    WvT_r = WvT.rearrange("(k p) c -> p k c", p=P)
    W2T_r = W2T.rearrange("(k p) c -> p k c", p=P)

    with tile.TileContext(nc) as tc:
        from contextlib import ExitStack

        with ExitStack() as ctx:
            wpool = ctx.enter_context(tc.tile_pool(name="weights", bufs=8))
            apool = ctx.enter_context(tc.tile_pool(name="acts", bufs=1))
            lpool = ctx.enter_context(tc.tile_pool(name="logits", bufs=MH))
            qwpool = ctx.enter_context(tc.tile_pool(name="qw", bufs=MH))
            const = ctx.enter_context(tc.tile_pool(name="const", bufs=1))
            stage = ctx.enter_context(tc.tile_pool(name="stage", bufs=6))
            b0pool = ctx.enter_context(tc.tile_pool(name="b0stage", bufs=8))
            psum = ctx.enter_context(
                tc.tile_pool(name="psum", bufs=o["psum_bufs"], space="PSUM"))

            # Consts packed into one DMA: bv | bq | b2eff | wh
            cst = const.tile([P, 3 * 16 + QROW], F32)

            def dma_cst():
                nc.sync.dma_start(out=cst[:], in_=constC)
            bv_sb = cst[:, 0:16]
            bq_sb = cst[:, 16:32]
            b2_sb = cst[:, 32:48]
            wh_sb = cst[:, 48:48 + QROW]

            if o["warmup"]:
                wup = stage.tile([P, 64], BF16, tag="wup", name="wup")
                nc.vector.memset(wup[:], 0.0)
                wps = psum.tile([64, 64], F32, tag="ps", name="pswarm")
                for _ in range(o["warmup"]):
                    nc.tensor.matmul(wps[:], lhsT=wup[:, 0:64], rhs=wup[:],
                                     start=True, stop=True)

            # SBUF tiles (allocation order is not DMA order)
            vtn = [apool.tile([P, KV, NT], BF16, name=f"vt{n}") for n in range(NN)]
            qt_all = apool.tile([P, KQ, QROW], BF16)
            wq_cb = H // o["wq_split"]
            wqts = [wpool.tile([P, KQ, wq_cb], BF16, tag="w", name=f"wq{s}")
                    for s in range(o["wq_split"])]
            wv_cb = H // o["wv_split"]
            wvts = [wpool.tile([P, KV, wv_cb], BF16, tag="w", name=f"wv{s}")
                    for s in range(o["wv_split"])]
            w2_cb = VD // o["w2_split"]
            w2ts = [wpool.tile([P, KH, w2_cb], BF16, tag="w", name=f"w2{s}")
                    for s in range(o["w2_split"])]

            def dma_vt(n, k0=0, k1=KV):
                nc.sync.dma_start(out=vtn[n][:, k0:k1, :], in_=vT_r[:, n, k0:k1, :])

            def dma_qt():
                nc.sync.dma_start(out=qt_all[:], in_=qT_r)

            def dma_wq(s, k0=0, k1=KQ):
                e = {"sync": nc.sync, "gpsimd": nc.gpsimd,
                     "scalar": nc.scalar}[o["wq_eng"]]
                e.dma_start(out=wqts[s][:, k0:k1, :],
                            in_=WqT_r[:, k0:k1, s * wq_cb:(s + 1) * wq_cb])

            def dma_wv(s, k0=0, k1=KV):
                nc.sync.dma_start(out=wvts[s][:, k0:k1, :],
                                  in_=WvT_r[:, k0:k1, s * wv_cb:(s + 1) * wv_cb])

            def dma_w2(s):
                nc.sync.dma_start(out=w2ts[s][:],
                                  in_=W2T_r[:, :, s * w2_cb:(s + 1) * w2_cb])

            # DMA emission order == HWDGE dispatch order == transfer order.
            # Hand-paced: each chunk lands just before the PE stream needs it
            # (PE order: warmup, B-b0 (m0-3, n0 then n1, ACT-only evictions),
            #  A halves, deferred b0 qw-multiplies, B blocks m4-15, C).
            if o["b0_first"]:
                ck = KV // o["wv0_chunks"]
                dma_vt(0, 0, 8)
                for c in range(0, 8 // ck):
                    dma_wv(0, c * ck, (c + 1) * ck)
                dma_vt(0, 8, 16)
                for c in range(8 // ck, KV // ck):
                    dma_wv(0, c * ck, (c + 1) * ck)
                dma_cst()
                dma_vt(1, 0, 8)
                dma_vt(1, 8, 16)
                dma_qt()
                dma_wq(0, 0, 4)
                dma_wq(0, 4, 8)
                dma_wq(1, 0, 4)
                dma_wq(1, 4, 8)
                dma_wv(1, 0, 8)
                dma_wv(1, 8, 16)
            else:
                dma_cst()
                dma_qt()
                dma_vt(0, 0, 8)
                dma_wv(0, 0, 4)
                dma_vt(0, 8, 16)
                dma_wv(0, 4, 8)
                dma_wv(0, 8, 12)
                dma_wv(0, 12, 16)
                dma_wq(0, 0, 4)
                dma_wq(0, 4, 8)
                dma_wq(1, 0, 4)
                dma_wq(1, 4, 8)
                dma_vt(1)
                dma_wv(1)
            for s in range(2, o["wv_split"]):
                dma_wv(s)
            for s in range(o["w2_split"]):
                dma_w2(s)

            def wq_lhsT(k, m):
                s, r = divmod(m * P, wq_cb)
                return wqts[s][:, k, r:r + P]

            def wv_lhsT(k, m):
                s, r = divmod(m * P, wv_cb)
                return wvts[s][:, k, r:r + P]

            def w2_lhsT(k, m):
                s, r = divmod(m * P, w2_cb)
                return w2ts[s][:, k, r:r + P]

            lts = [None] * MH
            qwts = [None] * MH

            def b_matmuls(groups, pss):
                for k in range(KV):
                    for (m, n) in groups:
                        nc.tensor.matmul(
                            pss[(m, n)][:], lhsT=wv_lhsT(k, m),
                            rhs=vtn[n][:, k, :],
                            start=(k == 0), stop=(k == KV - 1))

            def b_evict(m, n, ps):
                vs = stage.tile([P, NT], F32, tag="vstage", name=f"vs{m}_{n}")
                nc.scalar.activation(vs[:], ps[:],
                                     mybir.ActivationFunctionType.Relu,
                                     bias=bv_sb[:, m:m + 1])
                qb = qwts[m][:, n * BPT:(n + 1) * BPT].to_broadcast([P, BPT, NO])
                nc.vector.tensor_mul(
                    lts[m][:, n * NT:(n + 1) * NT].rearrange(
                        "p (b o) -> p b o", b=BPT),
                    vs.rearrange("p (b o) -> p b o", b=BPT), qb)

            def a_block(ms):
                pss = {m: psum.tile([P, QROW], F32, tag="ps", name=f"psA{m}")
                       for m in ms}
                for k in range(KQ):
                    for m in ms:
                        nc.tensor.matmul(
                            pss[m][:], lhsT=wq_lhsT(k, m), rhs=qt_all[:, k, :],
                            start=(k == 0), stop=(k == KQ - 1))
                for m in ms:
                    qs = stage.tile([P, QROW], F32, tag="qstage", name=f"qs{m}")
                    nc.scalar.activation(qs[:], pss[m][:],
                                         mybir.ActivationFunctionType.Relu,
                                         bias=bq_sb[:, m:m + 1])
                    qp = stage.tile([P, QROW], F32, tag="qstage", name=f"qp{m}")
                    nc.vector.tensor_mul(qp[:], qs[:], wh_sb)
                    qw = qwpool.tile([P, BS], F32, tag="qw", name=f"qw{m}")
                    nc.vector.tensor_reduce(
                        qw[:], qp.rearrange("p (b q) -> p b q", b=BS),
                        axis=mybir.AxisListType.X, op=mybir.AluOpType.add)
                    qwts[m] = qw

            if o["b0_first"]:
                # B-block0 (m0-3), n=0 then n=1: matmuls + ACT relu now (the
                # relu frees the PSUM banks); the qw multiplies are deferred
                # until phase A has produced qw. This front-loads 15.4us of
                # real PE work that only needs vT + the first WvT column
                # block, while the WqT stream is still on the bus.
                for m in range(4):
                    lts[m] = lpool.tile([P, NROW], BF16, tag="lt", name=f"lt{m}")
                b0_vs = {}
                for n in range(NN):
                    g0 = [(m, n) for m in range(4)]
                    pss0 = {(m, n): psum.tile([P, NT], F32, tag="ps",
                                              name=f"psB{m}_{n}")
                            for m in range(4)}
                    b_matmuls(g0, pss0)
                    for m in range(4):
                        vs = b0pool.tile([P, NT], F32, tag="b0s",
                                         name=f"b0vs{m}_{n}")
                        nc.scalar.activation(vs[:], pss0[(m, n)][:],
                                             mybir.ActivationFunctionType.Relu,
                                             bias=bv_sb[:, m:m + 1])
                        b0_vs[(m, n)] = vs
                # Phase A in halves (b0's banks were released by the relus).
                for half in range(2):
                    a_block(list(range(half * 8, half * 8 + 8)))
                for (m, n), vs in b0_vs.items():
                    qb = qwts[m][:, n * BPT:(n + 1) * BPT].to_broadcast(
                        [P, BPT, NO])
                    nc.vector.tensor_mul(
                        lts[m][:, n * NT:(n + 1) * NT].rearrange(
                            "p (b o) -> p b o", b=BPT),
                        vs.rearrange("p (b o) -> p b o", b=BPT), qb)
                rest_blocks = [list(range(4, 8)), list(range(8, 12)),
                               list(range(12, 16))]
            else:
                for half in range(2):
                    a_block(list(range(half * 8, half * 8 + 8)))
                rest_blocks = [list(range(0, 4)), list(range(4, 8)),
                               list(range(8, 12)), list(range(12, 16))]

            for ms in rest_blocks:
                for m in ms:
                    lts[m] = lpool.tile([P, NROW], BF16, tag="lt", name=f"lt{m}")
                groups = [(m, n) for m in ms for n in range(NN)]
                pss = {(m, n): psum.tile([P, NT], F32, tag="ps", name=f"psB{m}_{n}")
                       for (m, n) in groups}
                b_matmuls(groups, pss)
                for (m, n) in groups:
                    b_evict(m, n, pss[(m, n)])

            # ---- Phase C: out_T[vd, n] = W2 @ logits + b2eff
            eng_out = {"sync": nc.sync, "scalar": nc.scalar}[o["out_eng"]]
            for m in range(MV):
                os_ = stage.tile([P, NROW], F32, tag="ostage", name=f"os{m}")
                for n in range(NN):
                    # Split the very last group in half so the kernel-tail
                    # evict->DMA chain runs on a half-width tile.
                    last = (m == MV - 1 and n == NN - 1)
                    nsub = 2 if (last and o["tail_split"]) else 1
                    w = NT // nsub
                    for h in range(nsub):
                        c0 = n * NT + h * w
                        ps = psum.tile([P, w], F32, tag="ps",
                                       name=f"psC{m}_{n}_{h}")
                        for k in range(KH):
                            nc.tensor.matmul(
                                ps[:], lhsT=w2_lhsT(k, m),
                                rhs=lts[k][:, c0:c0 + w],
                                start=(k == 0), stop=(k == KH - 1))
                        nc.scalar.activation(os_[:, c0:c0 + w], ps[:],
                                             mybir.ActivationFunctionType.Identity,
                                             bias=b2_sb[:, m:m + 1])
                        if o["out_split"]:
                            eng_out.dma_start(
                                out=outT[m * P:(m + 1) * P, c0:c0 + w],
                                in_=os_[:, c0:c0 + w])
                if not o["out_split"]:
                    eng_out.dma_start(
                        out=outT[m * P:(m + 1) * P, :], in_=os_[:])

    nc.compile()
    return nc


_NC_CACHE = {}


def get_program(opts=None):
    key = tuple(sorted(opts.items())) if opts else ()
    if key not in _NC_CACHE:
        _NC_CACHE[key] = _build_program(opts)
    return _NC_CACHE[key]


def make_in_maps(v, q, Wv, bv, Wq, bq, wh, bh, W2, b2):
    """Host-side prep: shard batch, pre-transpose, pre-cast."""
    WvT = np.ascontiguousarray(Wv.astype(BF16_NP).T)           # [VD, H]
    WqT = np.ascontiguousarray(Wq.astype(BF16_NP).T)           # [QD, H]
    W2T = np.ascontiguousarray(W2.astype(BF16_NP).T)           # [H, VD]
    b2eff = (b2.astype(np.float64)
             + float(bh) * W2.astype(np.float64).sum(axis=1)).astype(np.float32)
    constC = np.zeros((P, 3 * 16 + QROW), np.float32)
    constC[:, 0:16] = bv.astype(np.float32).reshape(MH, P).T
    constC[:, 16:32] = bq.astype(np.float32).reshape(MH, P).T
    constC[:, 32:48] = b2eff.reshape(MV, P).T
    constC[:, 48:] = np.tile(wh.astype(np.float32), BS)[None, :]

    in_maps = []
    for c in range(NCORES):
        b0 = c * BS
        v_sh = v[b0:b0 + BS].reshape(NROW, VD).astype(BF16_NP)
        q_sh = q[b0:b0 + BS].reshape(QROW, QD).astype(BF16_NP)
        # vT: [P, n, k, c] flattened; qT: [P, k, c] flattened (k-major rows
        # contiguous per partition for single-descriptor DMAs)
        vT_c = (v_sh.T.reshape(KV, P, NN, NT).transpose(1, 2, 0, 3)
                .reshape(P, NN * KV * NT))
        qT_c = q_sh.T.reshape(KQ, P, QROW).transpose(1, 0, 2).reshape(P, KQ * QROW)
        in_maps.append({
            "vT": np.ascontiguousarray(vT_c),
            "qT": np.ascontiguousarray(qT_c),
            "WvT": WvT, "WqT": WqT, "W2T": W2T,
            "constC": constC,
        })
    return in_maps


def assemble_output(results):
    outs = []
    for c in range(NCORES):
        outT = results[c]["outT"]                      # [VD, NROW] f32
        outs.append(np.ascontiguousarray(outT.T).reshape(BS, NO, VD))
    return np.concatenate(outs, axis=0)


def kernel(v, q, Wv, bv, Wq, bq, wh, bh, W2, b2, **_unused):
    v, q, Wv, bv, Wq, bq, wh, bh, W2, b2 = (
        np.asarray(x) for x in (v, q, Wv, bv, Wq, bq, wh, bh, W2, b2))
    nc = get_program()
    in_maps = make_in_maps(v, q, Wv, bv, Wq, bq, wh, bh, W2, b2)
    res = run_bass_kernel_spmd(nc, in_maps, list(range(NCORES)))
    return assemble_output(res.results)



# revision 2
# speedup vs baseline: 1.2691x; 1.2691x over previous
"""Trainium2 Bass kernel for BCNet-style fused block — fp8 hi/lo split.

Reference computation (per batch b):
    v_ = relu(v @ Wv.T + bv)            # [B, NO, H]
    q_ = relu(q @ Wq.T + bq)            # [B, Q,  H]
    qw = einsum("bqh,q->bh", q_, wh)    # [B, H]
    logits = v_ * qw[:, None, :] + bh   # [B, NO, H]
    out = logits @ W2.T + b2            # [B, NO, VD]

Strategy: pure data parallel over batch (16 per core x 8 cores), weights
replicated. Every matmul operand x is split x = x_hi + x_lo (both fp8 e4m3,
power-of-2 pre-scaling so values sit in the normal range), and each logical
matmul A@B runs as fp8 DoubleRow instructions:
  - main: one instruction per k-tile PAIR computing Ah_k0@Bh_k0 + Ah_k1@Bh_k1
  - corr: one instruction per k-tile computing  Ah_k@Bl_k + Al_k@Bh_k
All accumulate into the same fp32 PSUM group, so per logical matmul the PE
does 1.5 k-passes of DoubleRow work = 0.75x the bf16 cost, with quantization
error ~0.1% per matmul (lo*lo term dropped).

Scale folding: v,q scaled by 4; Wv,Wq,W2 by 64; logits by 4 (folded into wh
on host). PSUM values are 256x the true values; evictions apply
activation(scale=1/256, bias=...). bh is folded into b2eff on host.

SBUF sub-layouts (s = hi/lo index):
  weights  [P, s(hi,lo), k, cols]   acts  [P, s(lo,hi), k, cols]
so a correction instruction's lhsT = w[:, 0:2, k, m-slice] pairs with
rhs = a[:, 0:2, k, n-slice] to give exactly (Wh@Al + Wl@Ah).

Phases (PE order): warmup -> B matmuls m0-7 (relu-only evictions stashed)
-> A (q-path) + qw -> deferred logit muls for m0-7 -> B m8-15 inline
-> C (out = logits8 @ W2split). DMA is hand-paced on the sync queue.
"""

import os
import sys

import numpy as np

for _p in ("/opt/trn_rl_repo", "/root/.axon_site/_ro/trn_rl_repo"):
    if os.path.isdir(_p) and _p not in sys.path:
        sys.path.insert(0, _p)

import ml_dtypes

import concourse.bacc as bacc
import concourse.bass as bass
import concourse.mybir as mybir
import concourse.tile as tile
from concourse.bass_utils import run_bass_kernel_spmd

B, NO, Q = 128, 36, 14
VD, QD, H = 2048, 1024, 2048
NCORES = 8
BS = B // NCORES          # 16 batches per core
NROW = BS * NO            # 576 v-rows per core
QROW = BS * Q             # 224 q-rows per core
P = 128
NT = 144                  # n-tile (4 batches * 36); DoubleRow rhs free=288<=512
NN = NROW // NT           # 4
BPT = NT // NO            # 4 batches per n-tile
KV = VD // P              # 16 contraction tiles for matmul 1
KQ = QD // P              # 8  contraction tiles for matmul 2
MH = H // P               # 16 output h-tiles
KH = H // P               # 16 contraction tiles for matmul 3
MV = VD // P              # 16 output vd-tiles

F32 = mybir.dt.float32
BF16 = mybir.dt.bfloat16
FP8 = mybir.dt.float8e4
E4_NP = ml_dtypes.float8_e4m3
BF16_NP = ml_dtypes.bfloat16
DR = mybir.MatmulPerfMode.DoubleRow

SV = 4.0     # activation scale (v, q)
SW = 64.0    # weight scale (Wv, Wq, W2)
SL = 4.0     # logits scale (folded into wh on host)
INV = 1.0 / 256.0   # eviction scale: 1/(SV*SW) = 1/(SL*SW)

WV_CB = 512          # Wv/W2 column-block width -> 4 blocks, 16KB tiles
WQ_CB = 1024         # Wq column-block width -> 2 blocks, 16KB tiles


def _build_program(opts=None):
    o = dict(
        warmup=70,
        wv_kchunk=4,      # k-tiles per DMA chunk within a Wv/W2 block
        wv0_ck=4,         # finer chunking for the first Wv block
        tail_split=True,  # split last output DMA per n-tile
        kint=16,          # k-tiles per interleaved (main+corr) sub-chunk
        b1_order="nm",    # first-half B group order: n-major or m-major
        b2_order="nm",    # second-half B group order
        out_eng="sync",   # queue for output DMAs ("sync" or "scalar")
    )
    if opts:
        o.update(opts)

    nc = bacc.Bacc("TRN2", target_bir_lowering=False, debug=False,
                   num_devices=NCORES)

    # DRAM tensors (all pre-split/interleaved on host)
    vT = nc.dram_tensor("vT", [P, NN * 2 * KV * NT], FP8,
                        kind="ExternalInput").ap()
    qT = nc.dram_tensor("qT", [P, 2 * KQ * QROW], FP8,
                        kind="ExternalInput").ap()
    WvT = nc.dram_tensor("WvT", [P, 2 * KV * H], FP8,
                         kind="ExternalInput").ap()
    WqT = nc.dram_tensor("WqT", [P, 2 * KQ * H], FP8,
                         kind="ExternalInput").ap()
    W2T = nc.dram_tensor("W2T", [P, 2 * KH * VD], FP8,
                         kind="ExternalInput").ap()
    constC = nc.dram_tensor("constC", [P, 3 * 16 + QROW], F32,
                            kind="ExternalInput").ap()
    outT = nc.dram_tensor("outT", [VD, NROW], F32, kind="ExternalOutput").ap()

    vT_r = vT.rearrange("p (n s k c) -> p n s k c", n=NN, s=2, k=KV)
    qT_r = qT.rearrange("p (s k c) -> p s k c", s=2, k=KQ)
    WvT_r = WvT.rearrange("p (s k c) -> p s k c", s=2, k=KV)
    WqT_r = WqT.rearrange("p (s k c) -> p s k c", s=2, k=KQ)
    W2T_r = W2T.rearrange("p (s k c) -> p s k c", s=2, k=KH)

    NWV = H // WV_CB      # 4
    NWQ = H // WQ_CB      # 2
    NW2 = VD // WV_CB     # 4

    with tile.TileContext(nc) as tc:
        from contextlib import ExitStack

        with ExitStack() as ctx:
            wpool = ctx.enter_context(tc.tile_pool(name="weights", bufs=7))
            apool = ctx.enter_context(tc.tile_pool(name="acts", bufs=1))
            lpool = ctx.enter_context(tc.tile_pool(name="logits", bufs=1))
            qwpool = ctx.enter_context(tc.tile_pool(name="qw", bufs=MH))
            const = ctx.enter_context(tc.tile_pool(name="const", bufs=1))
            stage = ctx.enter_context(tc.tile_pool(name="stage", bufs=6))
            vspool = ctx.enter_context(tc.tile_pool(name="vstash", bufs=40))
            lfpool = ctx.enter_context(tc.tile_pool(name="lf", bufs=6))
            ospool = ctx.enter_context(tc.tile_pool(name="ostage", bufs=3))
            psum = ctx.enter_context(
                tc.tile_pool(name="psum", bufs=8, space="PSUM"))

            # Consts packed into one DMA: bv | bq | b2eff | wh_eff
            cst = const.tile([P, 3 * 16 + QROW], F32)
            bv_sb = cst[:, 0:16]
            bq_sb = cst[:, 16:32]
            b2_sb = cst[:, 32:48]
            wh_sb = cst[:, 48:48 + QROW]

            if o["warmup"]:
                wup = stage.tile([P, 64], BF16, tag="wup", name="wup")
                nc.vector.memset(wup[:], 0.0)
                wps = psum.tile([64, 64], F32, tag="ps", name="pswarm")
                for _ in range(o["warmup"]):
                    nc.tensor.matmul(wps[:], lhsT=wup[:, 0:64], rhs=wup[:],
                                     start=True, stop=True)

            # SBUF tiles
            vt = apool.tile([P, NN, 2, KV, NT], FP8, name="vt")
            qt = apool.tile([P, 2, KQ, QROW], FP8, name="qt")
            lts = lpool.tile([P, 2, KH, NROW], FP8, name="lts")
            wvts = [wpool.tile([P, 2, KV, WV_CB], FP8, tag="w", name=f"wv{s}")
                    for s in range(NWV)]
            wqts = [wpool.tile([P, 2, KQ, WQ_CB], FP8, tag="w", name=f"wq{s}")
                    for s in range(NWQ)]
            w2ts = [wpool.tile([P, 2, KH, WV_CB], FP8, tag="w", name=f"w2{s}")
                    for s in range(NW2)]

            # ---- DMA helpers (sync queue; emission order == transfer order)
            def dma_cst():
                nc.sync.dma_start(out=cst[:], in_=constC)

            def dma_v(n):
                nc.sync.dma_start(out=vt[:, n], in_=vT_r[:, n])

            def dma_vp(n, s, k0, k1):
                nc.sync.dma_start(out=vt[:, n, s, k0:k1, :],
                                  in_=vT_r[:, n, s, k0:k1, :])

            def dma_q():
                nc.sync.dma_start(out=qt[:], in_=qT_r)

            def dma_wv(s, sub, k0, k1):
                nc.sync.dma_start(
                    out=wvts[s][:, sub, k0:k1, :],
                    in_=WvT_r[:, sub, k0:k1, s * WV_CB:(s + 1) * WV_CB])

            def dma_wq(s, sub):
                nc.sync.dma_start(
                    out=wqts[s][:, sub],
                    in_=WqT_r[:, sub, :, s * WQ_CB:(s + 1) * WQ_CB])

            def dma_w2(s, sub):
                nc.sync.dma_start(
                    out=w2ts[s][:, sub],
                    in_=W2T_r[:, sub, :, s * WV_CB:(s + 1) * WV_CB])

            def dma_wv_block(s):
                for (s_, sub, k0, k1) in wv_chunks(s):
                    dma_wv(s_, sub, k0, k1)

            # DMA stream order (hand-paced to PE consumption). Wv blocks are
            # emitted in (sub, k-chunk) pieces interleaved with the v n-tiles
            # so B's k-interleaved groups can start as chunks land.
            kint = o["kint"]

            def wv_chunks(s):
                # (sub, k) pieces in the order B's k-interleaved groups
                # consume them: per kint-range, hi chunks then lo chunks.
                ck = o["wv0_ck"] if s == 0 else o["wv_kchunk"]
                for kc in range(0, KV, kint):
                    for sub in range(2):
                        for c in range(kc, kc + kint, ck):
                            yield (s, sub, c, c + ck)

            dma_cst()
            # batch (m0-3, n0-1): per window, v hi pieces + wv hi chunks,
            # then v lo pieces + wv lo chunks (matching win-major PE order)
            ck0 = o["wv0_ck"]
            for kc in range(0, KV, kint):
                for sub_pe, sub_v in ((0, 1), (1, 0)):   # w-hi/v-hi, w-lo/v-lo
                    dma_vp(0, sub_v, kc, kc + kint)
                    dma_vp(1, sub_v, kc, kc + kint)
                    for c in range(kc, kc + kint, ck0):
                        dma_wv(0, sub_pe, c, c + ck0)
            # batch (m0-3, n2-3): v n2, n3 pieces in window order
            for kc in range(0, KV, kint):
                for sub_v in (1, 0):
                    dma_vp(2, sub_v, kc, kc + kint)
                    dma_vp(3, sub_v, kc, kc + kint)
            dma_wv_block(1)       # B m4-7
            dma_q()
            dma_wq(0, 0)          # A m0-7 (hi then lo)
            dma_wq(0, 1)
            dma_wq(1, 0)          # A m8-15
            dma_wq(1, 1)
            dma_wv_block(2)       # B m8-11
            dma_wv_block(3)       # B m12-15
            for s in range(NW2):  # C
                dma_w2(s, 0)
                dma_w2(s, 1)

            # ---- matmul slice helpers
            def wv_main(k0, m):
                s, r = divmod(m * P, WV_CB)
                return wvts[s][:, 0, k0:k0 + 2, r:r + P]

            def wv_corr(k, m):
                s, r = divmod(m * P, WV_CB)
                return wvts[s][:, 0:2, k, r:r + P]

            def wq_main(k0, m):
                s, r = divmod(m * P, WQ_CB)
                return wqts[s][:, 0, k0:k0 + 2, r:r + P]

            def wq_corr(k, m):
                s, r = divmod(m * P, WQ_CB)
                return wqts[s][:, 0:2, k, r:r + P]

            def w2_main(k0, m):
                s, r = divmod(m * P, WV_CB)
                return w2ts[s][:, 0, k0:k0 + 2, r:r + P]

            def w2_corr(k, m):
                s, r = divmod(m * P, WV_CB)
                return w2ts[s][:, 0:2, k, r:r + P]

            def split_group(ps, wmain, wcorr, rmain, rcorr, nk, ki=None):
                """Emit one full hi/lo-split accumulation group into psum ps.

                wmain(k0) / rmain(k0): 2-k-tile hi slices; wcorr(k)/rcorr(k):
                (hi,lo)x(lo,hi) 1-k-tile pair slices. nk = # 128-k-tiles.
                ki: k-interleave granularity (mains then corrs per ki-range),
                matching the (hi,lo)-per-ki-range DMA chunk order.
                """
                ki = ki or nk
                for kc in range(0, nk, ki):
                    for k0 in range(kc, kc + ki, 2):
                        nc.tensor.matmul(ps[:], lhsT=wmain(k0), rhs=rmain(k0),
                                         start=(k0 == 0), stop=False,
                                         perf_mode=DR)
                    for k in range(kc, kc + ki):
                        nc.tensor.matmul(ps[:], lhsT=wcorr(k), rhs=rcorr(k),
                                         start=False, stop=(k == nk - 1),
                                         perf_mode=DR)

            qwts = [None] * MH
            vstash = {}

            def b_relu(m, n, ps):
                vs = vspool.tile([P, NT], F32, tag="vs", name=f"vs{m}_{n}")
                nc.scalar.activation(vs[:], ps[:],
                                     mybir.ActivationFunctionType.Relu,
                                     bias=bv_sb[:, m:m + 1], scale=INV)
                vstash[(m, n)] = vs

            def b_batch(groups):
                """Win-major B matmuls across up to 8 (m, n) groups.

                Per kint-window: mains of every group, then corrs of every
                group — matching the DMA chunk order, so PE never blocks on
                one group's next window while another group's data is ready.
                """
                pss = {g: psum.tile([P, NT], F32, tag="ps",
                                    name=f"psB{g[0]}_{g[1]}")
                       for g in groups}
                for kc in range(0, KV, kint):
                    for (m, n) in groups:
                        for k0 in range(kc, kc + kint, 2):
                            nc.tensor.matmul(
                                pss[(m, n)][:], lhsT=wv_main(k0, m),
                                rhs=vt[:, n, 1, k0:k0 + 2, :],
                                start=(k0 == 0), stop=False, perf_mode=DR)
                    for (m, n) in groups:
                        for k in range(kc, kc + kint):
                            nc.tensor.matmul(
                                pss[(m, n)][:], lhsT=wv_corr(k, m),
                                rhs=vt[:, n, 0:2, k, :],
                                start=False, stop=(k == KV - 1),
                                perf_mode=DR)
                return pss

            def b_group(m, n):
                """Phase-B matmuls for tile (m, n) + relu eviction to stash."""
                ps = psum.tile([P, NT], F32, tag="ps", name=f"psB{m}_{n}")
                split_group(
                    ps,
                    lambda k0: wv_main(k0, m), lambda k: wv_corr(k, m),
                    lambda k0: vt[:, n, 1, k0:k0 + 2, :],
                    lambda k: vt[:, n, 0:2, k, :],
                    KV, ki=kint)
                b_relu(m, n, ps)

            def b_evict(m, n):
                """Deferred logit production: lf = vs*qb; lh, ll -> lts."""
                vs = vstash.pop((m, n))
                lf = lfpool.tile([P, NT], F32, tag="lf", name=f"lf{m}_{n}")
                qb = qwts[m][:, n * BPT:(n + 1) * BPT].to_broadcast(
                    [P, BPT, NO])
                nc.vector.tensor_mul(
                    lf.rearrange("p (b o) -> p b o", b=BPT),
                    vs.rearrange("p (b o) -> p b o", b=BPT), qb)
                nsl = slice(n * NT, (n + 1) * NT)
                nc.gpsimd.tensor_copy(lts[:, 1, m, nsl], lf[:])
                nc.vector.tensor_sub(lts[:, 0, m, nsl], lf[:],
                                     lts[:, 1, m, nsl])

            def a_group(m):
                ps = psum.tile([P, QROW], F32, tag="ps", name=f"psA{m}")
                split_group(
                    ps,
                    lambda k0: wq_main(k0, m), lambda k: wq_corr(k, m),
                    lambda k0: qt[:, 1, k0:k0 + 2, :],
                    lambda k: qt[:, 0:2, k, :],
                    KQ)
                qs = stage.tile([P, QROW], F32, tag="qstage", name=f"qs{m}")
                nc.scalar.activation(qs[:], ps[:],
                                     mybir.ActivationFunctionType.Relu,
                                     bias=bq_sb[:, m:m + 1], scale=INV)
                qp = stage.tile([P, QROW], F32, tag="qstage", name=f"qp{m}")
                nc.vector.tensor_mul(qp[:], qs[:], wh_sb)
                qw = qwpool.tile([P, BS], F32, tag="qw", name=f"qw{m}")
                nc.vector.tensor_reduce(
                    qw[:], qp.rearrange("p (b q) -> p b q", b=BS),
                    axis=mybir.AxisListType.X, op=mybir.AluOpType.add)
                qwts[m] = qw

            def group_order(ms, mode):
                if mode == "nm":
                    return [(m, n) for n in range(NN) for m in ms]
                return [(m, n) for m in ms for n in range(NN)]

            # ---- Phase B first half (m0-7): win-major batches, relu only
            for ms, ns in (((0, 1, 2, 3), (0, 1)), ((0, 1, 2, 3), (2, 3)),
                           ((4, 5, 6, 7), (0, 1)), ((4, 5, 6, 7), (2, 3))):
                pss = b_batch([(m, n) for n in ns for m in ms])
                for (m, n), ps in pss.items():
                    b_relu(m, n, ps)
            # ---- Phase A
            for m in range(MH):
                a_group(m)
            # ---- deferred logit evictions for m0-7
            for m in range(8):
                for n in range(NN):
                    b_evict(m, n)
            # ---- Phase B second half (m8-15): inline evictions
            for m, n in group_order(range(8, MH), o["b2_order"]):
                b_group(m, n)
                b_evict(m, n)

            # ---- Phase C: outT[vd, :] = (lts_hi+lts_lo) @ W2split + b2eff
            eng_out = {"sync": nc.sync, "scalar": nc.scalar}[o["out_eng"]]
            for m in range(MV):
                os_ = ospool.tile([P, NROW], F32, tag="os", name=f"os{m}")
                for n in range(NN):
                    nsl = slice(n * NT, (n + 1) * NT)
                    ps = psum.tile([P, NT], F32, tag="ps", name=f"psC{m}_{n}")
                    split_group(
                        ps,
                        lambda k0: w2_main(k0, m), lambda k: w2_corr(k, m),
                        lambda k0: lts[:, 1, k0:k0 + 2, nsl],
                        lambda k: lts[:, 0:2, k, nsl],
                        KH)
                    nc.scalar.activation(
                        os_[:, nsl], ps[:],
                        mybir.ActivationFunctionType.Identity,
                        bias=b2_sb[:, m:m + 1], scale=INV)
                    last = (m == MV - 1)
                    if last and o["tail_split"]:
                        eng_out.dma_start(
                            out=outT[m * P:(m + 1) * P, nsl], in_=os_[:, nsl])
                if not (last and o["tail_split"]):
                    eng_out.dma_start(out=outT[m * P:(m + 1) * P, :],
                                      in_=os_[:])

    nc.compile()
    return nc


_NC_CACHE = {}


def get_program(opts=None):
    key = tuple(sorted(opts.items())) if opts else ()
    if key not in _NC_CACHE:
        _NC_CACHE[key] = _build_program(opts)
    return _NC_CACHE[key]


def _split8(x):
    """x (f32) -> (hi, lo) fp8 e4m3 with x ~= hi + lo."""
    hi = x.astype(E4_NP)
    lo = (x - hi.astype(np.float32)).astype(E4_NP)
    return hi, lo


def _prep_weight(W, scale, kt):
    """W.T scaled+split -> [P, 2(hi,lo), kt, cols] flattened per partition."""
    WT = np.ascontiguousarray(W.astype(np.float32).T) * scale  # [K, M]
    K, M = WT.shape
    hi, lo = _split8(WT)
    arr = np.stack([hi, lo])                 # [2, K, M]
    arr = arr.reshape(2, kt, P, M).transpose(2, 0, 1, 3)   # [P, 2, kt, M]
    return np.ascontiguousarray(arr.reshape(P, 2 * kt * M))


def make_in_maps(v, q, Wv, bv, Wq, bq, wh, bh, W2, b2):
    """Host-side prep: shard batch, scale, split to fp8 hi/lo, interleave."""
    WvT8 = _prep_weight(Wv, SW, KV)          # [P, 2*16*2048]
    WqT8 = _prep_weight(Wq, SW, KQ)          # [P, 2*8*2048]
    W2T8 = _prep_weight(W2, SW, KH)          # [P, 2*16*2048]
    b2eff = (b2.astype(np.float64)
             + float(bh) * W2.astype(np.float64).sum(axis=1)).astype(np.float32)
    constC = np.zeros((P, 3 * 16 + QROW), np.float32)
    constC[:, 0:16] = bv.astype(np.float32).reshape(MH, P).T
    constC[:, 16:32] = bq.astype(np.float32).reshape(MH, P).T
    constC[:, 32:48] = b2eff.reshape(MV, P).T
    constC[:, 48:] = np.tile(wh.astype(np.float32) * SL, BS)[None, :]

    in_maps = []
    for c in range(NCORES):
        b0 = c * BS
        v_sh = v[b0:b0 + BS].reshape(NROW, VD).astype(np.float32) * SV
        q_sh = q[b0:b0 + BS].reshape(QROW, QD).astype(np.float32) * SV
        # vT: [P, n, s(lo,hi), k, NT]
        vhi, vlo = _split8(np.ascontiguousarray(v_sh.T))     # [VD, NROW]
        va = np.stack([vlo, vhi])                            # [2, VD, NROW]
        va = (va.reshape(2, KV, P, NN, NT)
              .transpose(2, 3, 0, 1, 4))                     # [P, n, 2, k, NT]
        qhi, qlo = _split8(np.ascontiguousarray(q_sh.T))     # [QD, QROW]
        qa = np.stack([qlo, qhi])                            # [2, QD, QROW]
        qa = (qa.reshape(2, KQ, P, QROW)
              .transpose(2, 0, 1, 3))                        # [P, 2, k, QROW]
        in_maps.append({
            "vT": np.ascontiguousarray(va.reshape(P, NN * 2 * KV * NT)),
            "qT": np.ascontiguousarray(qa.reshape(P, 2 * KQ * QROW)),
            "WvT": WvT8, "WqT": WqT8, "W2T": W2T8,
            "constC": constC,
        })
    return in_maps


def assemble_output(results):
    outs = []
    for c in range(NCORES):
        outT = results[c]["outT"]                      # [VD, NROW] f32
        outs.append(np.ascontiguousarray(outT.T).reshape(BS, NO, VD))
    return np.concatenate(outs, axis=0)


def kernel(v, q, Wv, bv, Wq, bq, wh, bh, W2, b2, **_unused):
    v, q, Wv, bv, Wq, bq, wh, bh, W2, b2 = (
        np.asarray(x) for x in (v, q, Wv, bv, Wq, bq, wh, bh, W2, b2))
    nc = get_program()
    in_maps = make_in_maps(v, q, Wv, bv, Wq, bq, wh, bh, W2, b2)
    res = run_bass_kernel_spmd(nc, in_maps, list(range(NCORES)))
    return assemble_output(res.results)


# revision 3
# speedup vs baseline: 1.3192x; 1.0395x over previous
"""Trainium2 Bass kernel for BCNet-style fused block — fp8 hi/lo split.

Reference computation (per batch b):
    v_ = relu(v @ Wv.T + bv)            # [B, NO, H]
    q_ = relu(q @ Wq.T + bq)            # [B, Q,  H]
    qw = einsum("bqh,q->bh", q_, wh)    # [B, H]
    logits = v_ * qw[:, None, :] + bh   # [B, NO, H]
    out = logits @ W2.T + b2            # [B, NO, VD]

Strategy: pure data parallel over batch (16 per core x 8 cores), weights
replicated. Every matmul operand x is split x = x_hi + x_lo (both fp8 e4m3,
power-of-2 pre-scaling so values sit in the normal range), and each logical
matmul A@B runs as fp8 DoubleRow instructions:
  - main: one instruction per k-tile PAIR computing Ah_k0@Bh_k0 + Ah_k1@Bh_k1
  - corr: one instruction per k-tile computing  Ah_k@Bl_k + Al_k@Bh_k
All accumulate into the same fp32 PSUM group, so per logical matmul the PE
does 1.5 k-passes of DoubleRow work = 0.75x the bf16 cost, with quantization
error ~0.1% per matmul (lo*lo term dropped).

Scale folding: v,q scaled by 4; Wv,Wq,W2 by 64; logits by 4 (folded into wh
on host). PSUM values are 256x the true values; evictions apply
activation(scale=1/256, bias=...). bh is folded into b2eff on host.

SBUF sub-layouts (s = hi/lo index):
  weights  [P, s(hi,lo), k, cols]   acts  [P, s(lo,hi), k, cols]
so a correction instruction's lhsT = w[:, 0:2, k, m-slice] pairs with
rhs = a[:, 0:2, k, n-slice] to give exactly (Wh@Al + Wl@Ah).

Phases (PE order): warmup -> B matmuls m0-7 (relu-only evictions stashed)
-> A (q-path) + qw -> deferred logit muls for m0-7 -> B m8-15 inline
-> C (out = logits8 @ W2split). DMA is hand-paced on the sync queue.
"""

import os
import sys

import numpy as np

for _p in ("/opt/trn_rl_repo", "/root/.axon_site/_ro/trn_rl_repo"):
    if os.path.isdir(_p) and _p not in sys.path:
        sys.path.insert(0, _p)

import ml_dtypes

import concourse.bacc as bacc
import concourse.bass as bass
import concourse.mybir as mybir
import concourse.tile as tile
from concourse.bass_utils import run_bass_kernel_spmd

B, NO, Q = 128, 36, 14
VD, QD, H = 2048, 1024, 2048
NCORES = 8
BS = B // NCORES          # 16 batches per core
NROW = BS * NO            # 576 v-rows per core
QROW = BS * Q             # 224 q-rows per core
P = 128
NT = 144                  # n-tile (4 batches * 36); DoubleRow rhs free=288<=512
NN = NROW // NT           # 4
BPT = NT // NO            # 4 batches per n-tile
KV = VD // P              # 16 contraction tiles for matmul 1
KQ = QD // P              # 8  contraction tiles for matmul 2
MH = H // P               # 16 output h-tiles
KH = H // P               # 16 contraction tiles for matmul 3
MV = VD // P              # 16 output vd-tiles

F32 = mybir.dt.float32
BF16 = mybir.dt.bfloat16
FP8 = mybir.dt.float8e4
E4_NP = ml_dtypes.float8_e4m3
BF16_NP = ml_dtypes.bfloat16
DR = mybir.MatmulPerfMode.DoubleRow

SV = 4.0     # activation scale (v, q)
SW = 64.0    # weight scale (Wv, Wq, W2)
SL = 4.0     # logits scale (folded into wh on host)
INV = 1.0 / 256.0   # eviction scale: 1/(SV*SW) = 1/(SL*SW)

WV_CB = 512          # Wv/W2 column-block width -> 4 blocks, 16KB tiles
WQ_CB = 1024         # Wq column-block width -> 2 blocks, 16KB tiles


def _build_program(opts=None):
    o = dict(
        warmup=70,
        wv_kchunk=8,      # k-tiles per DMA chunk within a Wv/W2 block
        wv0_ck=4,         # finer chunking for the first Wv block
        tail_split=2,     # sub-splits of the final output group
        kint=16,          # k-tiles per interleaved (main+corr) sub-chunk
        kint0=16,         # window granularity for the first batch + block 0
        b1_order="nm",    # first-half B group order: n-major or m-major
        b2_order="nm",    # second-half B group order
        out_eng="alt",    # output DMA queues: sync/scalar/alt(sync+vector)
        skip1=2,          # corr k-tiles skipped (from top) in MM1
        skip3=0,          # corr k-tiles skipped (from top) in MM3
    )
    if opts:
        o.update(opts)

    nc = bacc.Bacc("TRN2", target_bir_lowering=False, debug=False,
                   num_devices=NCORES)

    # DRAM tensors (all pre-split/interleaved on host)
    vT = nc.dram_tensor("vT", [P, NN * 2 * KV * NT], FP8,
                        kind="ExternalInput").ap()
    qT = nc.dram_tensor("qT", [P, 2 * KQ * QROW], FP8,
                        kind="ExternalInput").ap()
    WvT = nc.dram_tensor("WvT", [P, 2 * KV * H], FP8,
                         kind="ExternalInput").ap()
    WqT = nc.dram_tensor("WqT", [P, 2 * KQ * H], FP8,
                         kind="ExternalInput").ap()
    W2T = nc.dram_tensor("W2T", [P, 2 * KH * VD], FP8,
                         kind="ExternalInput").ap()
    constC = nc.dram_tensor("constC", [P, 3 * 16 + QROW], F32,
                            kind="ExternalInput").ap()
    outT = nc.dram_tensor("outT", [VD, NROW], F32, kind="ExternalOutput").ap()

    vT_r = vT.rearrange("p (n s k c) -> p n s k c", n=NN, s=2, k=KV)
    qT_r = qT.rearrange("p (s k c) -> p s k c", s=2, k=KQ)
    WvT_r = WvT.rearrange("p (s k c) -> p s k c", s=2, k=KV)
    WqT_r = WqT.rearrange("p (s k c) -> p s k c", s=2, k=KQ)
    W2T_r = W2T.rearrange("p (s k c) -> p s k c", s=2, k=KH)

    NWV = H // WV_CB      # 4
    NWQ = H // WQ_CB      # 2
    NW2 = VD // WV_CB     # 4

    with tile.TileContext(nc) as tc:
        from contextlib import ExitStack

        with ExitStack() as ctx:
            wpool = ctx.enter_context(tc.tile_pool(name="weights", bufs=7))
            apool = ctx.enter_context(tc.tile_pool(name="acts", bufs=1))
            lpool = ctx.enter_context(tc.tile_pool(name="logits", bufs=1))
            qwpool = ctx.enter_context(tc.tile_pool(name="qw", bufs=MH))
            const = ctx.enter_context(tc.tile_pool(name="const", bufs=1))
            stage = ctx.enter_context(tc.tile_pool(name="stage", bufs=6))
            vspool = ctx.enter_context(tc.tile_pool(name="vstash", bufs=40))
            lfpool = ctx.enter_context(tc.tile_pool(name="lf", bufs=6))
            ospool = ctx.enter_context(tc.tile_pool(name="ostage", bufs=12))
            psum = ctx.enter_context(
                tc.tile_pool(name="psum", bufs=8, space="PSUM"))

            # Consts packed into one DMA: bv | bq | b2eff | wh_eff
            cst = const.tile([P, 3 * 16 + QROW], F32)
            bv_sb = cst[:, 0:16]
            bq_sb = cst[:, 16:32]
            b2_sb = cst[:, 32:48]
            wh_sb = cst[:, 48:48 + QROW]

            if o["warmup"]:
                wup = stage.tile([P, 64], BF16, tag="wup", name="wup")
                nc.gpsimd.memset(wup[:], 0.0)
                wps = psum.tile([64, 64], F32, tag="ps", name="pswarm")
                for _ in range(o["warmup"]):
                    nc.tensor.matmul(wps[:], lhsT=wup[:, 0:64], rhs=wup[:],
                                     start=True, stop=True)

            # SBUF tiles
            vt = apool.tile([P, NN, 2, KV, NT], FP8, name="vt")
            qt = apool.tile([P, 2, KQ, QROW], FP8, name="qt")
            lts = lpool.tile([P, 2, KH, NROW], FP8, name="lts")
            wvts = [wpool.tile([P, 2, KV, WV_CB], FP8, tag="w", name=f"wv{s}")
                    for s in range(NWV)]
            wqts = [wpool.tile([P, 2, KQ, WQ_CB], FP8, tag="w", name=f"wq{s}")
                    for s in range(NWQ)]
            w2ts = [wpool.tile([P, 2, KH, WV_CB], FP8, tag="w", name=f"w2{s}")
                    for s in range(NW2)]

            # ---- DMA helpers (sync queue; emission order == transfer order)
            def dma_cst():
                nc.sync.dma_start(out=cst[:], in_=constC)

            def dma_v(n):
                nc.sync.dma_start(out=vt[:, n], in_=vT_r[:, n])

            def dma_vp(n, s, k0, k1):
                nc.sync.dma_start(out=vt[:, n, s, k0:k1, :],
                                  in_=vT_r[:, n, s, k0:k1, :])

            def dma_q():
                nc.sync.dma_start(out=qt[:], in_=qT_r)

            def dma_wv(s, sub, k0, k1):
                nc.sync.dma_start(
                    out=wvts[s][:, sub, k0:k1, :],
                    in_=WvT_r[:, sub, k0:k1, s * WV_CB:(s + 1) * WV_CB])

            def dma_wq(s, sub):
                nc.sync.dma_start(
                    out=wqts[s][:, sub],
                    in_=WqT_r[:, sub, :, s * WQ_CB:(s + 1) * WQ_CB])

            def dma_w2(s, sub):
                nc.sync.dma_start(
                    out=w2ts[s][:, sub],
                    in_=W2T_r[:, sub, :, s * WV_CB:(s + 1) * WV_CB])

            def dma_wv_block(s):
                for (s_, sub, k0, k1) in wv_chunks(s, lo_kmax=KV - o["skip1"]):
                    dma_wv(s_, sub, k0, k1)

            # DMA stream order (hand-paced to PE consumption). Wv blocks are
            # emitted in (sub, k-chunk) pieces interleaved with the v n-tiles
            # so B's k-interleaved groups can start as chunks land.
            kint = o["kint"]

            def wv_chunks(s, lo_kmax=None):
                # (sub, k) pieces in the order B's k-interleaved groups
                # consume them: per kint-range, hi chunks then lo chunks.
                # lo-sub chunks above lo_kmax are never read (skipped corrs).
                ck = o["wv0_ck"] if s == 0 else o["wv_kchunk"]
                lo_kmax = KV if lo_kmax is None else lo_kmax
                for kc in range(0, KV, kint):
                    for sub in range(2):
                        for c in range(kc, kc + kint, ck):
                            c1 = min(c + ck, lo_kmax) if sub == 1 else c + ck
                            if c1 > c:
                                yield (s, sub, c, c1)

            # batch (m0-3, n0-1): per window, v hi pieces + wv hi chunks,
            # then v lo pieces + wv lo chunks (matching win-major PE order).
            # cst rides after the first wv chunk (first needed by B relu).
            ck0 = o["wv0_ck"]
            kint0 = o["kint0"]
            first_chunk = True
            for kc in range(0, KV, kint0):
                for sub_pe, sub_v in ((0, 1), (1, 0)):   # w-hi/v-hi, w-lo/v-lo
                    dma_vp(0, sub_v, kc, kc + kint0)
                    dma_vp(1, sub_v, kc, kc + kint0)
                    for c in range(kc, kc + kint0, ck0):
                        dma_wv(0, sub_pe, c, c + ck0)
                        if first_chunk:
                            dma_cst()
                            first_chunk = False
            # batch (m0-3, n2-3): v n2, n3 pieces in window order
            for kc in range(0, KV, kint):
                for sub_v in (1, 0):
                    dma_vp(2, sub_v, kc, kc + kint)
                    dma_vp(3, sub_v, kc, kc + kint)
            dma_wv_block(1)       # B m4-7
            dma_q()
            dma_wq(0, 0)          # A m0-7 (hi then lo)
            dma_wq(0, 1)
            dma_wq(1, 0)          # A m8-15
            dma_wq(1, 1)
            dma_wv_block(2)       # B m8-11
            dma_wv_block(3)       # B m12-15
            for s in range(NW2):  # C
                dma_w2(s, 0)
                dma_w2(s, 1)

            # ---- matmul slice helpers
            def wv_main(k0, m):
                s, r = divmod(m * P, WV_CB)
                return wvts[s][:, 0, k0:k0 + 2, r:r + P]

            def wv_corr(k, m):
                s, r = divmod(m * P, WV_CB)
                return wvts[s][:, 0:2, k, r:r + P]

            def wq_main(k0, m):
                s, r = divmod(m * P, WQ_CB)
                return wqts[s][:, 0, k0:k0 + 2, r:r + P]

            def wq_corr(k, m):
                s, r = divmod(m * P, WQ_CB)
                return wqts[s][:, 0:2, k, r:r + P]

            def w2_main(k0, m):
                s, r = divmod(m * P, WV_CB)
                return w2ts[s][:, 0, k0:k0 + 2, r:r + P]

            def w2_corr(k, m):
                s, r = divmod(m * P, WV_CB)
                return w2ts[s][:, 0:2, k, r:r + P]

            def split_group(ps, wmain, wcorr, rmain, rcorr, nk, ki=None,
                            skip=0):
                """Emit one full hi/lo-split accumulation group into psum ps.

                wmain(k0) / rmain(k0): 2-k-tile hi slices; wcorr(k)/rcorr(k):
                (hi,lo)x(lo,hi) 1-k-tile pair slices. nk = # 128-k-tiles.
                ki: k-interleave granularity (mains then corrs per ki-range),
                matching the (hi,lo)-per-ki-range DMA chunk order.
                skip: drop the corr instructions for the top `skip` k-tiles
                (spends error budget for PE time).
                """
                ki = ki or nk
                klast = nk - 1 - skip
                for kc in range(0, nk, ki):
                    for k0 in range(kc, kc + ki, 2):
                        nc.tensor.matmul(ps[:], lhsT=wmain(k0), rhs=rmain(k0),
                                         start=(k0 == 0), stop=False,
                                         perf_mode=DR)
                    for k in range(kc, kc + ki):
                        if k > klast:
                            continue
                        nc.tensor.matmul(ps[:], lhsT=wcorr(k), rhs=rcorr(k),
                                         start=False, stop=(k == klast),
                                         perf_mode=DR)

            qwts = [None] * MH
            vstash = {}

            def b_relu(m, n, ps):
                vs = vspool.tile([P, NT], F32, tag="vs", name=f"vs{m}_{n}")
                nc.scalar.activation(vs[:], ps[:],
                                     mybir.ActivationFunctionType.Relu,
                                     bias=bv_sb[:, m:m + 1], scale=INV)
                vstash[(m, n)] = vs

            def b_batch(groups, ki=None):
                """Win-major B matmuls across up to 8 (m, n) groups.

                Per kint-window: mains of every group, then corrs of every
                group — matching the DMA chunk order, so PE never blocks on
                one group's next window while another group's data is ready.
                """
                ki = ki or kint
                klast = KV - 1 - o["skip1"]
                pss = {g: psum.tile([P, NT], F32, tag="ps",
                                    name=f"psB{g[0]}_{g[1]}")
                       for g in groups}
                for kc in range(0, KV, ki):
                    for (m, n) in groups:
                        for k0 in range(kc, kc + ki, 2):
                            nc.tensor.matmul(
                                pss[(m, n)][:], lhsT=wv_main(k0, m),
                                rhs=vt[:, n, 1, k0:k0 + 2, :],
                                start=(k0 == 0), stop=False, perf_mode=DR)
                    for (m, n) in groups:
                        for k in range(kc, kc + ki):
                            if k > klast:
                                continue
                            nc.tensor.matmul(
                                pss[(m, n)][:], lhsT=wv_corr(k, m),
                                rhs=vt[:, n, 0:2, k, :],
                                start=False, stop=(k == klast),
                                perf_mode=DR)
                return pss

            def b_group(m, n):
                """Phase-B matmuls for tile (m, n) + relu eviction to stash."""
                ps = psum.tile([P, NT], F32, tag="ps", name=f"psB{m}_{n}")
                split_group(
                    ps,
                    lambda k0: wv_main(k0, m), lambda k: wv_corr(k, m),
                    lambda k0: vt[:, n, 1, k0:k0 + 2, :],
                    lambda k: vt[:, n, 0:2, k, :],
                    KV, ki=kint, skip=o["skip1"])
                b_relu(m, n, ps)

            def b_evict(m, n):
                """Deferred logit production: lf = vs*qb; lh, ll -> lts."""
                vs = vstash.pop((m, n))
                lf = lfpool.tile([P, NT], F32, tag="lf", name=f"lf{m}_{n}")
                qb = qwts[m][:, n * BPT:(n + 1) * BPT].to_broadcast(
                    [P, BPT, NO])
                nc.vector.tensor_mul(
                    lf.rearrange("p (b o) -> p b o", b=BPT),
                    vs.rearrange("p (b o) -> p b o", b=BPT), qb)
                nsl = slice(n * NT, (n + 1) * NT)
                nc.gpsimd.tensor_copy(lts[:, 1, m, nsl], lf[:])
                if m < KH - o["skip3"]:
                    nc.vector.tensor_sub(lts[:, 0, m, nsl], lf[:],
                                         lts[:, 1, m, nsl])

            def a_group(m):
                ps = psum.tile([P, QROW], F32, tag="ps", name=f"psA{m}")
                split_group(
                    ps,
                    lambda k0: wq_main(k0, m), lambda k: wq_corr(k, m),
                    lambda k0: qt[:, 1, k0:k0 + 2, :],
                    lambda k: qt[:, 0:2, k, :],
                    KQ)
                qs = stage.tile([P, QROW], F32, tag="qstage", name=f"qs{m}")
                nc.scalar.activation(qs[:], ps[:],
                                     mybir.ActivationFunctionType.Relu,
                                     bias=bq_sb[:, m:m + 1], scale=INV)
                qp = stage.tile([P, QROW], F32, tag="qstage", name=f"qp{m}")
                nc.vector.tensor_mul(qp[:], qs[:], wh_sb)
                qw = qwpool.tile([P, BS], F32, tag="qw", name=f"qw{m}")
                nc.vector.tensor_reduce(
                    qw[:], qp.rearrange("p (b q) -> p b q", b=BS),
                    axis=mybir.AxisListType.X, op=mybir.AluOpType.add)
                qwts[m] = qw

            def group_order(ms, mode):
                if mode == "nm":
                    return [(m, n) for n in range(NN) for m in ms]
                return [(m, n) for m in ms for n in range(NN)]

            # ---- Phase B first half (m0-7): win-major batches, relu only
            first_batch = True
            for ms, ns in (((0, 1, 2, 3), (0, 1)), ((0, 1, 2, 3), (2, 3)),
                           ((4, 5, 6, 7), (0, 1)), ((4, 5, 6, 7), (2, 3))):
                pss = b_batch([(m, n) for n in ns for m in ms],
                              ki=o["kint0"] if first_batch else None)
                first_batch = False
                for (m, n), ps in pss.items():
                    b_relu(m, n, ps)
            # ---- Phase A
            for m in range(MH):
                a_group(m)
            # ---- deferred logit evictions for m0-7
            for m in range(8):
                for n in range(NN):
                    b_evict(m, n)
            # ---- Phase B second half (m8-15): inline evictions
            for m, n in group_order(range(8, MH), o["b2_order"]):
                b_group(m, n)
                b_evict(m, n)

            # ---- Phase C: outT[vd, :] = (lts_hi+lts_lo) @ W2split + b2eff
            # n-major so the n3 groups (whose logits evict last) come with
            # maximal slack; one output DMA per (m, n) piece.
            out_engs = {"sync": [nc.sync], "scalar": [nc.scalar],
                        "alt": [nc.sync, nc.gpsimd],
                        "alt3": [nc.sync, nc.gpsimd, nc.scalar]}[o["out_eng"]]
            for n in range(NN):
                nsl = slice(n * NT, (n + 1) * NT)
                for m in range(MV):
                    last = (m == MV - 1 and n == NN - 1)
                    nsub = o["tail_split"] if last else 1
                    w = NT // nsub
                    for h in range(nsub):
                        c0 = n * NT + h * w
                        hsl = slice(c0, c0 + w)
                        ps = psum.tile([P, w], F32, tag="ps",
                                       name=f"psC{m}_{n}_{h}")
                        split_group(
                            ps,
                            lambda k0: w2_main(k0, m),
                            lambda k: w2_corr(k, m),
                            lambda k0: lts[:, 1, k0:k0 + 2, hsl],
                            lambda k: lts[:, 0:2, k, hsl],
                            KH, skip=o["skip3"])
                        os_ = ospool.tile([P, w], F32, tag="os",
                                          name=f"os{m}_{n}_{h}")
                        nc.scalar.activation(
                            os_[:], ps[:],
                            mybir.ActivationFunctionType.Identity,
                            bias=b2_sb[:, m:m + 1], scale=INV)
                        eng = out_engs[(n * MV + m + h) % len(out_engs)]
                        eng.dma_start(
                            out=outT[m * P:(m + 1) * P, hsl], in_=os_[:])

    nc.compile()
    return nc


_NC_CACHE = {}


def get_program(opts=None):
    key = tuple(sorted(opts.items())) if opts else ()
    if key not in _NC_CACHE:
        _NC_CACHE[key] = _build_program(opts)
    return _NC_CACHE[key]


def _split8(x):
    """x (f32) -> (hi, lo) fp8 e4m3 with x ~= hi + lo."""
    hi = x.astype(E4_NP)
    lo = (x - hi.astype(np.float32)).astype(E4_NP)
    return hi, lo


def _prep_weight(W, scale, kt):
    """W.T scaled+split -> [P, 2(hi,lo), kt, cols] flattened per partition."""
    WT = np.ascontiguousarray(W.astype(np.float32).T) * scale  # [K, M]
    K, M = WT.shape
    hi, lo = _split8(WT)
    arr = np.stack([hi, lo])                 # [2, K, M]
    arr = arr.reshape(2, kt, P, M).transpose(2, 0, 1, 3)   # [P, 2, kt, M]
    return np.ascontiguousarray(arr.reshape(P, 2 * kt * M))


def make_in_maps(v, q, Wv, bv, Wq, bq, wh, bh, W2, b2):
    """Host-side prep: shard batch, scale, split to fp8 hi/lo, interleave."""
    WvT8 = _prep_weight(Wv, SW, KV)          # [P, 2*16*2048]
    WqT8 = _prep_weight(Wq, SW, KQ)          # [P, 2*8*2048]
    W2T8 = _prep_weight(W2, SW, KH)          # [P, 2*16*2048]
    b2eff = (b2.astype(np.float64)
             + float(bh) * W2.astype(np.float64).sum(axis=1)).astype(np.float32)
    constC = np.zeros((P, 3 * 16 + QROW), np.float32)
    constC[:, 0:16] = bv.astype(np.float32).reshape(MH, P).T
    constC[:, 16:32] = bq.astype(np.float32).reshape(MH, P).T
    constC[:, 32:48] = b2eff.reshape(MV, P).T
    constC[:, 48:] = np.tile(wh.astype(np.float32) * SL, BS)[None, :]

    in_maps = []
    for c in range(NCORES):
        b0 = c * BS
        v_sh = v[b0:b0 + BS].reshape(NROW, VD).astype(np.float32) * SV
        q_sh = q[b0:b0 + BS].reshape(QROW, QD).astype(np.float32) * SV
        # vT: [P, n, s(lo,hi), k, NT]
        vhi, vlo = _split8(np.ascontiguousarray(v_sh.T))     # [VD, NROW]
        va = np.stack([vlo, vhi])                            # [2, VD, NROW]
        va = (va.reshape(2, KV, P, NN, NT)
              .transpose(2, 3, 0, 1, 4))                     # [P, n, 2, k, NT]
        qhi, qlo = _split8(np.ascontiguousarray(q_sh.T))     # [QD, QROW]
        qa = np.stack([qlo, qhi])                            # [2, QD, QROW]
        qa = (qa.reshape(2, KQ, P, QROW)
              .transpose(2, 0, 1, 3))                        # [P, 2, k, QROW]
        in_maps.append({
            "vT": np.ascontiguousarray(va.reshape(P, NN * 2 * KV * NT)),
            "qT": np.ascontiguousarray(qa.reshape(P, 2 * KQ * QROW)),
            "WvT": WvT8, "WqT": WqT8, "W2T": W2T8,
            "constC": constC,
        })
    return in_maps


def assemble_output(results):
    outs = []
    for c in range(NCORES):
        outT = results[c]["outT"]                      # [VD, NROW] f32
        outs.append(np.ascontiguousarray(outT.T).reshape(BS, NO, VD))
    return np.concatenate(outs, axis=0)


def kernel(v, q, Wv, bv, Wq, bq, wh, bh, W2, b2, **_unused):
    v, q, Wv, bv, Wq, bq, wh, bh, W2, b2 = (
        np.asarray(x) for x in (v, q, Wv, bv, Wq, bq, wh, bh, W2, b2))
    nc = get_program()
    in_maps = make_in_maps(v, q, Wv, bv, Wq, bq, wh, bh, W2, b2)
    res = run_bass_kernel_spmd(nc, in_maps, list(range(NCORES)))
    return assemble_output(res.results)


# revision 4
# speedup vs baseline: 1.3393x; 1.0152x over previous
"""Trainium2 Bass kernel for BCNet-style fused block — fp8 hi/lo split.

Reference computation (per batch b):
    v_ = relu(v @ Wv.T + bv)            # [B, NO, H]
    q_ = relu(q @ Wq.T + bq)            # [B, Q,  H]
    qw = einsum("bqh,q->bh", q_, wh)    # [B, H]
    logits = v_ * qw[:, None, :] + bh   # [B, NO, H]
    out = logits @ W2.T + b2            # [B, NO, VD]

Strategy: pure data parallel over batch (16 per core x 8 cores), weights
replicated. Every matmul operand x is split x = x_hi + x_lo (both fp8 e4m3,
power-of-2 pre-scaling so values sit in the normal range), and each logical
matmul A@B runs as fp8 DoubleRow instructions:
  - main: one instruction per k-tile PAIR computing Ah_k0@Bh_k0 + Ah_k1@Bh_k1
  - corr: one instruction per k-tile computing  Ah_k@Bl_k + Al_k@Bh_k
All accumulate into the same fp32 PSUM group, so per logical matmul the PE
does 1.5 k-passes of DoubleRow work = 0.75x the bf16 cost, with quantization
error ~0.1% per matmul (lo*lo term dropped). The corr instructions for
the top `skip1` k-tiles of MM1 are dropped entirely (spends part of the
2e-2 error budget for PE time; measured rel l2 1.4e-2 at skip1=3).

Scale folding: v,q scaled by 4; Wv,Wq,W2 by 64; logits by 4 (folded into wh
on host). PSUM values are 256x the true values; evictions apply
activation(scale=1/256, bias=...). bh is folded into b2eff on host.

SBUF sub-layouts (s = hi/lo index):
  weights  [P, s(hi,lo), k, cols]   acts  [P, s(lo,hi), k, cols]
so a correction instruction's lhsT = w[:, 0:2, k, m-slice] pairs with
rhs = a[:, 0:2, k, n-slice] to give exactly (Wh@Al + Wl@Ah).

Phases (PE order): warmup -> B matmuls m0-7 (relu-only evictions stashed)
-> A (q-path) + qw -> deferred logit muls for m0-7 -> B m8-15 inline
-> C (out = logits8 @ W2split). DMA is hand-paced on the sync queue.
"""

import os
import sys

import numpy as np

for _p in ("/opt/trn_rl_repo", "/root/.axon_site/_ro/trn_rl_repo"):
    if os.path.isdir(_p) and _p not in sys.path:
        sys.path.insert(0, _p)

import ml_dtypes

import concourse.bacc as bacc
import concourse.bass as bass
import concourse.mybir as mybir
import concourse.tile as tile
from concourse.bass_utils import run_bass_kernel_spmd

B, NO, Q = 128, 36, 14
VD, QD, H = 2048, 1024, 2048
NCORES = 8
BS = B // NCORES          # 16 batches per core
NROW = BS * NO            # 576 v-rows per core
QROW = BS * Q             # 224 q-rows per core
P = 128
NT = 144                  # n-tile (4 batches * 36); DoubleRow rhs free=288<=512
NN = NROW // NT           # 4
BPT = NT // NO            # 4 batches per n-tile
KV = VD // P              # 16 contraction tiles for matmul 1
KQ = QD // P              # 8  contraction tiles for matmul 2
MH = H // P               # 16 output h-tiles
KH = H // P               # 16 contraction tiles for matmul 3
MV = VD // P              # 16 output vd-tiles

F32 = mybir.dt.float32
BF16 = mybir.dt.bfloat16
FP8 = mybir.dt.float8e4
E4_NP = ml_dtypes.float8_e4m3
BF16_NP = ml_dtypes.bfloat16
DR = mybir.MatmulPerfMode.DoubleRow

SV = 4.0     # activation scale (v, q)
SW = 64.0    # weight scale (Wv, Wq, W2)
SL = 4.0     # logits scale (folded into wh on host)
INV = 1.0 / 256.0   # eviction scale: 1/(SV*SW) = 1/(SL*SW)

WV_CB = 512          # Wv/W2 column-block width -> 4 blocks, 16KB tiles
WQ_CB = 1024         # Wq column-block width -> 2 blocks, 16KB tiles


def _build_program(opts=None):
    o = dict(
        warmup=70,
        wv_kchunk=8,      # k-tiles per DMA chunk within a Wv/W2 block
        wv0_ck=8,         # finer chunking for the first Wv block
        tail_split=2,     # sub-splits of the final output group
        kint=16,          # k-tiles per interleaved (main+corr) sub-chunk
        kint0=16,         # window granularity for the first batch + block 0
        front_order="vfirst",  # block-0 stream: vfirst | vinter
        nt_c=144,         # phase-C n-tile width (must divide 576, <=256)
        tail_dve=0,       # evict final C groups on DVE instead of ACT
        last_scalar=1,    # issue the final out-DMA from the ACT queue
        b1_order="nm",    # first-half B group order: n-major or m-major
        b2_order="nm",    # second-half B group order
        out_eng="alt",    # output DMA queues: sync/scalar/alt(sync+vector)
        skip1=3,          # corr k-tiles skipped (from top) in MM1
        skip3=0,          # corr k-tiles skipped (from top) in MM3
    )
    if opts:
        o.update(opts)

    nc = bacc.Bacc("TRN2", target_bir_lowering=False, debug=False,
                   num_devices=NCORES)

    # DRAM tensors (all pre-split/interleaved on host)
    vT = nc.dram_tensor("vT", [P, NN * 2 * KV * NT], FP8,
                        kind="ExternalInput").ap()
    qT = nc.dram_tensor("qT", [P, 2 * KQ * QROW], FP8,
                        kind="ExternalInput").ap()
    WvT = nc.dram_tensor("WvT", [P, 2 * KV * H], FP8,
                         kind="ExternalInput").ap()
    WqT = nc.dram_tensor("WqT", [P, 2 * KQ * H], FP8,
                         kind="ExternalInput").ap()
    W2T = nc.dram_tensor("W2T", [P, 2 * KH * VD], FP8,
                         kind="ExternalInput").ap()
    constC = nc.dram_tensor("constC", [P, 3 * 16 + QROW], F32,
                            kind="ExternalInput").ap()
    outT = nc.dram_tensor("outT", [VD, NROW], F32, kind="ExternalOutput").ap()

    vT_r = vT.rearrange("p (n s k c) -> p n s k c", n=NN, s=2, k=KV)
    qT_r = qT.rearrange("p (s k c) -> p s k c", s=2, k=KQ)
    WvT_r = WvT.rearrange("p (s k c) -> p s k c", s=2, k=KV)
    WqT_r = WqT.rearrange("p (s k c) -> p s k c", s=2, k=KQ)
    W2T_r = W2T.rearrange("p (s k c) -> p s k c", s=2, k=KH)

    NWV = H // WV_CB      # 4
    NWQ = H // WQ_CB      # 2
    NW2 = VD // WV_CB     # 4

    with tile.TileContext(nc) as tc:
        from contextlib import ExitStack

        with ExitStack() as ctx:
            wpool = ctx.enter_context(tc.tile_pool(name="weights", bufs=7))
            apool = ctx.enter_context(tc.tile_pool(name="acts", bufs=1))
            lpool = ctx.enter_context(tc.tile_pool(name="logits", bufs=1))
            qwpool = ctx.enter_context(tc.tile_pool(name="qw", bufs=MH))
            const = ctx.enter_context(tc.tile_pool(name="const", bufs=1))
            stage = ctx.enter_context(tc.tile_pool(name="stage", bufs=6))
            vspool = ctx.enter_context(tc.tile_pool(name="vstash", bufs=40))
            lfpool = ctx.enter_context(tc.tile_pool(name="lf", bufs=6))
            ospool = ctx.enter_context(tc.tile_pool(name="ostage", bufs=12))
            psum = ctx.enter_context(
                tc.tile_pool(name="psum", bufs=8, space="PSUM"))

            # Consts packed into one DMA: bv | bq | b2eff | wh_eff
            cst = const.tile([P, 3 * 16 + QROW], F32)
            bv_sb = cst[:, 0:16]
            bq_sb = cst[:, 16:32]
            b2_sb = cst[:, 32:48]
            wh_sb = cst[:, 48:48 + QROW]

            if o["warmup"]:
                wup = stage.tile([P, 64], BF16, tag="wup", name="wup")
                nc.gpsimd.memset(wup[:], 0.0)
                wps = psum.tile([64, 64], F32, tag="ps", name="pswarm")
                for _ in range(o["warmup"]):
                    nc.tensor.matmul(wps[:], lhsT=wup[:, 0:64], rhs=wup[:],
                                     start=True, stop=True)

            # SBUF tiles
            vt = apool.tile([P, NN, 2, KV, NT], FP8, name="vt")
            qt = apool.tile([P, 2, KQ, QROW], FP8, name="qt")
            lts = lpool.tile([P, 2, KH, NROW], FP8, name="lts")
            wvts = [wpool.tile([P, 2, KV, WV_CB], FP8, tag="w", name=f"wv{s}")
                    for s in range(NWV)]
            wqts = [wpool.tile([P, 2, KQ, WQ_CB], FP8, tag="w", name=f"wq{s}")
                    for s in range(NWQ)]
            w2ts = [wpool.tile([P, 2, KH, WV_CB], FP8, tag="w", name=f"w2{s}")
                    for s in range(NW2)]

            # ---- DMA helpers (sync queue; emission order == transfer order)
            def dma_cst():
                nc.sync.dma_start(out=cst[:], in_=constC)

            def dma_v(n):
                nc.sync.dma_start(out=vt[:, n], in_=vT_r[:, n])

            def dma_vp(n, s, k0, k1):
                nc.sync.dma_start(out=vt[:, n, s, k0:k1, :],
                                  in_=vT_r[:, n, s, k0:k1, :])

            def dma_q():
                nc.sync.dma_start(out=qt[:], in_=qT_r)

            def dma_wv(s, sub, k0, k1):
                nc.sync.dma_start(
                    out=wvts[s][:, sub, k0:k1, :],
                    in_=WvT_r[:, sub, k0:k1, s * WV_CB:(s + 1) * WV_CB])

            def dma_wq(s, sub):
                nc.sync.dma_start(
                    out=wqts[s][:, sub],
                    in_=WqT_r[:, sub, :, s * WQ_CB:(s + 1) * WQ_CB])

            def dma_w2(s, sub):
                nc.sync.dma_start(
                    out=w2ts[s][:, sub],
                    in_=W2T_r[:, sub, :, s * WV_CB:(s + 1) * WV_CB])

            def dma_wv_block(s):
                for (s_, sub, k0, k1) in wv_chunks(s, lo_kmax=KV - o["skip1"]):
                    dma_wv(s_, sub, k0, k1)

            # DMA stream order (hand-paced to PE consumption). Wv blocks are
            # emitted in (sub, k-chunk) pieces interleaved with the v n-tiles
            # so B's k-interleaved groups can start as chunks land.
            kint = o["kint"]

            def wv_chunks(s, lo_kmax=None):
                # (sub, k) pieces in the order B's k-interleaved groups
                # consume them: per kint-range, hi chunks then lo chunks.
                # lo-sub chunks above lo_kmax are never read (skipped corrs).
                ck = o["wv0_ck"] if s == 0 else o["wv_kchunk"]
                lo_kmax = KV if lo_kmax is None else lo_kmax
                for kc in range(0, KV, kint):
                    for sub in range(2):
                        for c in range(kc, kc + kint, ck):
                            c1 = min(c + ck, lo_kmax) if sub == 1 else c + ck
                            if c1 > c:
                                yield (s, sub, c, c1)

            # batch (m0-3, n0-1): per window, v hi pieces + wv hi chunks,
            # then v lo pieces + wv lo chunks (matching win-major PE order).
            # cst rides after the first wv chunk (first needed by B relu).
            ck0 = o["wv0_ck"]
            kint0 = o["kint0"]
            first_chunk = True
            for kc in range(0, KV, kint0):
                for sub_pe, sub_v in ((0, 1), (1, 0)):   # w-hi/v-hi, w-lo/v-lo
                    dma_vp(0, sub_v, kc, kc + kint0)
                    dma_vp(1, sub_v, kc, kc + kint0)
                    for c in range(kc, kc + kint0, ck0):
                        dma_wv(0, sub_pe, c, c + ck0)
                        if first_chunk:
                            dma_cst()
                            first_chunk = False
            # batch (m0-3, n2-3): v n2, n3 pieces in window order
            for kc in range(0, KV, kint):
                for sub_v in (1, 0):
                    dma_vp(2, sub_v, kc, kc + kint)
                    dma_vp(3, sub_v, kc, kc + kint)
            dma_wv_block(1)       # B m4-7
            dma_q()
            dma_wq(0, 0)          # A m0-7 (hi then lo)
            dma_wq(0, 1)
            dma_wq(1, 0)          # A m8-15
            dma_wq(1, 1)
            dma_wv_block(2)       # B m8-11
            dma_wv_block(3)       # B m12-15
            for s in range(NW2):  # C
                dma_w2(s, 0)
                dma_w2(s, 1)

            # ---- matmul slice helpers
            def wv_main(k0, m):
                s, r = divmod(m * P, WV_CB)
                return wvts[s][:, 0, k0:k0 + 2, r:r + P]

            def wv_corr(k, m):
                s, r = divmod(m * P, WV_CB)
                return wvts[s][:, 0:2, k, r:r + P]

            def wq_main(k0, m):
                s, r = divmod(m * P, WQ_CB)
                return wqts[s][:, 0, k0:k0 + 2, r:r + P]

            def wq_corr(k, m):
                s, r = divmod(m * P, WQ_CB)
                return wqts[s][:, 0:2, k, r:r + P]

            def w2_main(k0, m):
                s, r = divmod(m * P, WV_CB)
                return w2ts[s][:, 0, k0:k0 + 2, r:r + P]

            def w2_corr(k, m):
                s, r = divmod(m * P, WV_CB)
                return w2ts[s][:, 0:2, k, r:r + P]

            def split_group(ps, wmain, wcorr, rmain, rcorr, nk, ki=None,
                            skip=0):
                """Emit one full hi/lo-split accumulation group into psum ps.

                wmain(k0) / rmain(k0): 2-k-tile hi slices; wcorr(k)/rcorr(k):
                (hi,lo)x(lo,hi) 1-k-tile pair slices. nk = # 128-k-tiles.
                ki: k-interleave granularity (mains then corrs per ki-range),
                matching the (hi,lo)-per-ki-range DMA chunk order.
                skip: drop the corr instructions for the top `skip` k-tiles
                (spends error budget for PE time).
                """
                ki = ki or nk
                klast = nk - 1 - skip
                for kc in range(0, nk, ki):
                    for k0 in range(kc, kc + ki, 2):
                        nc.tensor.matmul(ps[:], lhsT=wmain(k0), rhs=rmain(k0),
                                         start=(k0 == 0), stop=False,
                                         perf_mode=DR)
                    for k in range(kc, kc + ki):
                        if k > klast:
                            continue
                        nc.tensor.matmul(ps[:], lhsT=wcorr(k), rhs=rcorr(k),
                                         start=False, stop=(k == klast),
                                         perf_mode=DR)

            qwts = [None] * MH
            vstash = {}

            def b_relu(m, n, ps):
                vs = vspool.tile([P, NT], F32, tag="vs", name=f"vs{m}_{n}")
                nc.scalar.activation(vs[:], ps[:],
                                     mybir.ActivationFunctionType.Relu,
                                     bias=bv_sb[:, m:m + 1], scale=INV)
                vstash[(m, n)] = vs

            def b_batch(groups, ki=None):
                """Win-major B matmuls across up to 8 (m, n) groups.

                Per kint-window: mains of every group, then corrs of every
                group — matching the DMA chunk order, so PE never blocks on
                one group's next window while another group's data is ready.
                """
                ki = ki or kint
                klast = KV - 1 - o["skip1"]
                pss = {g: psum.tile([P, NT], F32, tag="ps",
                                    name=f"psB{g[0]}_{g[1]}")
                       for g in groups}
                for kc in range(0, KV, ki):
                    for (m, n) in groups:
                        for k0 in range(kc, kc + ki, 2):
                            nc.tensor.matmul(
                                pss[(m, n)][:], lhsT=wv_main(k0, m),
                                rhs=vt[:, n, 1, k0:k0 + 2, :],
                                start=(k0 == 0), stop=False, perf_mode=DR)
                    for (m, n) in groups:
                        for k in range(kc, kc + ki):
                            if k > klast:
                                continue
                            nc.tensor.matmul(
                                pss[(m, n)][:], lhsT=wv_corr(k, m),
                                rhs=vt[:, n, 0:2, k, :],
                                start=False, stop=(k == klast),
                                perf_mode=DR)
                return pss

            def b_group(m, n):
                """Phase-B matmuls for tile (m, n) + relu eviction to stash."""
                ps = psum.tile([P, NT], F32, tag="ps", name=f"psB{m}_{n}")
                split_group(
                    ps,
                    lambda k0: wv_main(k0, m), lambda k: wv_corr(k, m),
                    lambda k0: vt[:, n, 1, k0:k0 + 2, :],
                    lambda k: vt[:, n, 0:2, k, :],
                    KV, ki=kint, skip=o["skip1"])
                b_relu(m, n, ps)

            def b_evict(m, n):
                """Deferred logit production: lf = vs*qb; lh, ll -> lts."""
                vs = vstash.pop((m, n))
                lf = lfpool.tile([P, NT], F32, tag="lf", name=f"lf{m}_{n}")
                qb = qwts[m][:, n * BPT:(n + 1) * BPT].to_broadcast(
                    [P, BPT, NO])
                nc.vector.tensor_mul(
                    lf.rearrange("p (b o) -> p b o", b=BPT),
                    vs.rearrange("p (b o) -> p b o", b=BPT), qb)
                nsl = slice(n * NT, (n + 1) * NT)
                nc.gpsimd.tensor_copy(lts[:, 1, m, nsl], lf[:])
                if m < KH - o["skip3"]:
                    nc.vector.tensor_sub(lts[:, 0, m, nsl], lf[:],
                                         lts[:, 1, m, nsl])

            def a_group(m):
                ps = psum.tile([P, QROW], F32, tag="ps", name=f"psA{m}")
                split_group(
                    ps,
                    lambda k0: wq_main(k0, m), lambda k: wq_corr(k, m),
                    lambda k0: qt[:, 1, k0:k0 + 2, :],
                    lambda k: qt[:, 0:2, k, :],
                    KQ)
                qs = stage.tile([P, QROW], F32, tag="qstage", name=f"qs{m}")
                nc.scalar.activation(qs[:], ps[:],
                                     mybir.ActivationFunctionType.Relu,
                                     bias=bq_sb[:, m:m + 1], scale=INV)
                qp = stage.tile([P, QROW], F32, tag="qstage", name=f"qp{m}")
                nc.vector.tensor_mul(qp[:], qs[:], wh_sb)
                qw = qwpool.tile([P, BS], F32, tag="qw", name=f"qw{m}")
                nc.vector.tensor_reduce(
                    qw[:], qp.rearrange("p (b q) -> p b q", b=BS),
                    axis=mybir.AxisListType.X, op=mybir.AluOpType.add)
                qwts[m] = qw

            def group_order(ms, mode):
                if mode == "nm":
                    return [(m, n) for n in range(NN) for m in ms]
                return [(m, n) for m in ms for n in range(NN)]

            # ---- Phase B first half (m0-7): win-major batches, relu only
            first_batch = True
            for ms, ns in (((0, 1, 2, 3), (0, 1)), ((0, 1, 2, 3), (2, 3)),
                           ((4, 5, 6, 7), (0, 1)), ((4, 5, 6, 7), (2, 3))):
                pss = b_batch([(m, n) for n in ns for m in ms],
                              ki=o["kint0"] if first_batch else None)
                first_batch = False
                for (m, n), ps in pss.items():
                    b_relu(m, n, ps)
            # ---- Phase A
            for m in range(MH):
                a_group(m)
            # ---- deferred logit evictions for m0-7
            for m in range(8):
                for n in range(NN):
                    b_evict(m, n)
            # ---- Phase B second half (m8-15): inline evictions
            for m, n in group_order(range(8, MH), o["b2_order"]):
                b_group(m, n)
                b_evict(m, n)

            # ---- Phase C: outT[vd, :] = (lts_hi+lts_lo) @ W2split + b2eff
            # n-major so the n3 groups (whose logits evict last) come with
            # maximal slack; one output DMA per (m, n) piece.
            out_engs = {"sync": [nc.sync], "scalar": [nc.scalar],
                        "alt": [nc.sync, nc.gpsimd],
                        "alt3": [nc.sync, nc.gpsimd, nc.scalar]}[o["out_eng"]]
            NTC = o["nt_c"]
            NNC = NROW // NTC
            for n in range(NNC):
                for m in range(MV):
                    last = (m == MV - 1 and n == NNC - 1)
                    nsub = o["tail_split"] if last else 1
                    w = NTC // nsub
                    for h in range(nsub):
                        c0 = n * NTC + h * w
                        hsl = slice(c0, c0 + w)
                        ps = psum.tile([P, w], F32, tag="ps",
                                       name=f"psC{m}_{n}_{h}")
                        split_group(
                            ps,
                            lambda k0: w2_main(k0, m),
                            lambda k: w2_corr(k, m),
                            lambda k0: lts[:, 1, k0:k0 + 2, hsl],
                            lambda k: lts[:, 0:2, k, hsl],
                            KH, skip=o["skip3"])
                        os_ = ospool.tile([P, w], F32, tag="os",
                                          name=f"os{m}_{n}_{h}")
                        if last and o["tail_dve"]:
                            nc.vector.scalar_tensor_tensor(
                                os_[:], in0=ps[:], scalar=INV,
                                op0=mybir.AluOpType.mult,
                                op1=mybir.AluOpType.add,
                                in1=b2_sb[:, m:m + 1].to_broadcast([P, w]))
                        else:
                            nc.scalar.activation(
                                os_[:], ps[:],
                                mybir.ActivationFunctionType.Identity,
                                bias=b2_sb[:, m:m + 1], scale=INV)
                        if last and h == nsub - 1 and o["last_scalar"]:
                            eng = nc.scalar
                        else:
                            eng = out_engs[(n * MV + m + h) % len(out_engs)]
                        eng.dma_start(
                            out=outT[m * P:(m + 1) * P, hsl], in_=os_[:])

    nc.compile()
    return nc


_NC_CACHE = {}


def get_program(opts=None):
    key = tuple(sorted(opts.items())) if opts else ()
    if key not in _NC_CACHE:
        _NC_CACHE[key] = _build_program(opts)
    return _NC_CACHE[key]


def _split8(x):
    """x (f32) -> (hi, lo) fp8 e4m3 with x ~= hi + lo."""
    hi = x.astype(E4_NP)
    lo = (x - hi.astype(np.float32)).astype(E4_NP)
    return hi, lo


def _prep_weight(W, scale, kt):
    """W.T scaled+split -> [P, 2(hi,lo), kt, cols] flattened per partition."""
    WT = np.ascontiguousarray(W.astype(np.float32).T) * scale  # [K, M]
    K, M = WT.shape
    hi, lo = _split8(WT)
    arr = np.stack([hi, lo])                 # [2, K, M]
    arr = arr.reshape(2, kt, P, M).transpose(2, 0, 1, 3)   # [P, 2, kt, M]
    return np.ascontiguousarray(arr.reshape(P, 2 * kt * M))


def make_in_maps(v, q, Wv, bv, Wq, bq, wh, bh, W2, b2):
    """Host-side prep: shard batch, scale, split to fp8 hi/lo, interleave."""
    WvT8 = _prep_weight(Wv, SW, KV)          # [P, 2*16*2048]
    WqT8 = _prep_weight(Wq, SW, KQ)          # [P, 2*8*2048]
    W2T8 = _prep_weight(W2, SW, KH)          # [P, 2*16*2048]
    b2eff = (b2.astype(np.float64)
             + float(bh) * W2.astype(np.float64).sum(axis=1)).astype(np.float32)
    constC = np.zeros((P, 3 * 16 + QROW), np.float32)
    constC[:, 0:16] = bv.astype(np.float32).reshape(MH, P).T
    constC[:, 16:32] = bq.astype(np.float32).reshape(MH, P).T
    constC[:, 32:48] = b2eff.reshape(MV, P).T
    constC[:, 48:] = np.tile(wh.astype(np.float32) * SL, BS)[None, :]

    in_maps = []
    for c in range(NCORES):
        b0 = c * BS
        v_sh = v[b0:b0 + BS].reshape(NROW, VD).astype(np.float32) * SV
        q_sh = q[b0:b0 + BS].reshape(QROW, QD).astype(np.float32) * SV
        # vT: [P, n, s(lo,hi), k, NT]
        vhi, vlo = _split8(np.ascontiguousarray(v_sh.T))     # [VD, NROW]
        va = np.stack([vlo, vhi])                            # [2, VD, NROW]
        va = (va.reshape(2, KV, P, NN, NT)
              .transpose(2, 3, 0, 1, 4))                     # [P, n, 2, k, NT]
        qhi, qlo = _split8(np.ascontiguousarray(q_sh.T))     # [QD, QROW]
        qa = np.stack([qlo, qhi])                            # [2, QD, QROW]
        qa = (qa.reshape(2, KQ, P, QROW)
              .transpose(2, 0, 1, 3))                        # [P, 2, k, QROW]
        in_maps.append({
            "vT": np.ascontiguousarray(va.reshape(P, NN * 2 * KV * NT)),
            "qT": np.ascontiguousarray(qa.reshape(P, 2 * KQ * QROW)),
            "WvT": WvT8, "WqT": WqT8, "W2T": W2T8,
            "constC": constC,
        })
    return in_maps


def assemble_output(results):
    outs = []
    for c in range(NCORES):
        outT = results[c]["outT"]                      # [VD, NROW] f32
        outs.append(np.ascontiguousarray(outT.T).reshape(BS, NO, VD))
    return np.concatenate(outs, axis=0)


def kernel(v, q, Wv, bv, Wq, bq, wh, bh, W2, b2, **_unused):
    v, q, Wv, bv, Wq, bq, wh, bh, W2, b2 = (
        np.asarray(x) for x in (v, q, Wv, bv, Wq, bq, wh, bh, W2, b2))
    nc = get_program()
    in_maps = make_in_maps(v, q, Wv, bv, Wq, bq, wh, bh, W2, b2)
    res = run_bass_kernel_spmd(nc, in_maps, list(range(NCORES)))
    return assemble_output(res.results)
